# revision 7
# baseline (speedup 1.0000x reference)
"""Trainium2 Bass kernel for nn_HSNLayer (hypergraph message passing).

Computes, for COO adjacency A [N,N] and incidence B [N,E]:
    z1 = sigmoid((A @ x) @ W1_00)            # nodes -> nodes, level 1
    z2 = sigmoid((B^T @ x) @ W1_01)          # nodes -> edges, level 1
    out = sigmoid((A @ z1) @ W2_00 + (B @ z2) @ W2_10)
(uses SpMM/dense-matmul commutativity: A @ (x W) == (A @ x) W)

Strategy (8 NeuronCores, SPMD):
  - Destination rows sharded contiguously across cores; per-core COO streams
    sorted by (dest block of 128, source chunk of 32768) on host, padded to
    whole 128-row tiles.
  - Sparse gather of source rows via InstDMAGatherAnt (int16 idx, <=1024
    idxs per instruction -- SWDGE ring limit -- round-robined over 4 SWDGE
    queues for ~250 GB/s of 256B-row random gather).
  - Segment-sum by dest via selection-matrix matmul: for each 128-nnz tile,
    Msel[j,d] = val_j * (destlocal_j == d) built with two broadcast DVE ops,
    then PSUM accumulation of G_t^T @ Msel_t over each dest block.
  - Weight matrices folded in per dest block after the segment sum.
  - z1/z2 shards exchanged with AllGather collectives; level-2 phases gather
    from the all-gathered tables.
All gather-path data is fp16 (tolerance 2e-2; fp16 keeps ~1e-3 rel err).
"""
import numpy as np

import concourse.bacc as bacc
import concourse.mybir as mybir
import concourse.tile as tile

P = 128
C = 128
NCORES = 8
CHUNK = 32768
NODES = 100000
EDGES = 200000
NPAD = 102400     # 12800 per core
EPAD = 204800     # 25600 per core
NSHARD = NPAD // NCORES
ESHARD = EPAD // NCORES
XROWS = 4 * CHUNK  # padded x table
MAX_GATHER_TILES = 8   # 1024 idxs = SWDGE descriptor-ring capacity
NQUEUES = 4

F16 = mybir.dt.float16
F32 = mybir.dt.float32
I16 = mybir.dt.int16


# ---------------------------------------------------------------- host prep

def _cell_order(nb, nch, g):
    order = []
    for g0 in range(0, nb, g):
        for c_ in range(nch):
            for b_ in range(g0, min(g0 + g, nb)):
                order.append(b_ * nch + c_)
    return np.asarray(order)


def _prep_phase(dest, src, vals, dshard, nb, nch, g):
    """Sort/pad one phase's COO stream. Returns dict with shared tile counts T
    [nb,nch] (max over cores) and per-core stream tensors."""
    dest = np.asarray(dest, np.int64)
    src = np.asarray(src, np.int64)
    vals = np.asarray(vals, np.float32)
    core = dest // dshard
    dloc = dest % dshard
    b = dloc // P
    dl = dloc % P
    ch = src // CHUNK
    ix = src % CHUNK
    ncell = nb * nch
    cell = b * nch + ch
    gcell = core * ncell + cell
    cnt = np.bincount(gcell, minlength=NCORES * ncell).reshape(NCORES, nb, nch)
    T = -(-cnt.max(axis=0) // P)
    T[:, 0] = np.maximum(T[:, 0], 1)
    order_cells = _cell_order(nb, nch, g)
    Tflat = T.reshape(-1)
    in_order = Tflat[order_cells]
    off_in_order = np.concatenate([[0], np.cumsum(in_order)[:-1]])
    tile_off = np.empty(ncell, np.int64)
    tile_off[order_cells] = off_in_order
    total = int(Tflat.sum())

    order = np.argsort(gcell, kind="stable")
    gs = gcell[order]
    starts = np.searchsorted(gs, np.arange(NCORES * ncell))
    rank = np.arange(gs.size) - starts[gs]
    pos = tile_off[gs % ncell] * P + rank
    cs = gs // ncell
    idxf = np.zeros((NCORES, total * P), np.int16)
    dlf = np.zeros((NCORES, total * P), np.float16)
    vlf = np.zeros((NCORES, total * P), np.float16)
    idxf[cs, pos] = ix[order].astype(np.int16)
    dlf[cs, pos] = dl[order].astype(np.float16)
    vlf[cs, pos] = vals[order].astype(np.float16)

    idx16 = np.empty((NCORES, P, total * 8), np.int16)
    for k in range(NCORES):
        wrapped = idxf[k].reshape(total * 8, 16).T            # [16, total*8]
        idx16[k] = np.tile(wrapped, (8, 1))
    dl16 = np.ascontiguousarray(dlf.reshape(NCORES, total, P).transpose(0, 2, 1))
    vl16 = np.ascontiguousarray(vlf.reshape(NCORES, total, P).transpose(0, 2, 1))
    return dict(T=T, total=total, idx16=idx16, dl16=dl16, vl16=vl16,
                nb=nb, nch=nch, g=g)


# ---------------------------------------------------------------- program

class _Phase:
    """Emit gather + msel + accumulate-matmul stream for one phase."""

    def __init__(self, nc, pools, name, stream, idx_t, dl_t, vl_t, src_views, iota, zrhs, tagset=None):
        self.nc = nc
        self.pools = pools
        self.name = name
        self.tag = tagset if tagset is not None else name
        self.T = stream["T"]
        self.nb = stream["nb"]
        self.nch = stream["nch"]
        self.g = stream["g"]
        self.idx_t = idx_t
        self.dl_t = dl_t
        self.vl_t = vl_t
        self.src_views = src_views
        self.iota = iota
        self.zrhs = zrhs
        self.maxTg = max(
            int(self.T[g0:g0 + self.g, :].sum())
            for g0 in range(0, self.nb, self.g))
        self.maxTgc = max(
            int(self.T[g0:g0 + self.g, c].sum())
            for g0 in range(0, self.nb, self.g) for c in range(self.nch))

    def emit(self, post):
        nc = self.nc
        cpool, spool, ppool, _ = self.pools
        nm = self.tag
        qrr = 0
        tile_off = 0
        for g0 in range(0, self.nb, self.g):
            gb = list(range(g0, min(g0 + self.g, self.nb)))
            Tg = int(self.T[gb, :].sum())
            idxs = spool.tile([P, self.maxTg * 8], I16, tag=f"idx{nm}", name=f"idx{nm}")
            dls = spool.tile([P, self.maxTg], F16, tag=f"dl{nm}", name=f"dl{nm}")
            vls = spool.tile([P, self.maxTg], F16, tag=f"vl{nm}", name=f"vl{nm}")
            nc.sync.dma_start(out=idxs[:, :Tg * 8],
                              in_=self.idx_t[:, tile_off * 8:(tile_off + Tg) * 8])
            nc.sync.dma_start(out=dls[:, :Tg], in_=self.dl_t[:, tile_off:tile_off + Tg])
            nc.sync.dma_start(out=vls[:, :Tg], in_=self.vl_t[:, tile_off:tile_off + Tg])

            acc = ppool.tile([P, self.g * P], F32, tag="acc", name="acc")
            # PSUM start=True behaves per-bank: clear the whole accumulator
            # once with a zero matmul, then accumulate with start=False.
            nc.tensor.matmul(acc[:, :len(gb) * P], lhsT=self.iota[:],
                             rhs=self.zrhs[:, :len(gb) * P], start=True, stop=False)
            coff = 0
            for ch in range(self.nch):
                Tgc = int(self.T[gb, ch].sum())
                if Tgc == 0:
                    continue
                gbuf = spool.tile([P, self.maxTgc * C], F16, tag=f"gb{nm}", name=f"gb{nm}")
                msel = spool.tile([P, self.maxTgc * P], F16, tag=f"ms{nm}", name=f"ms{nm}")
                for s0 in range(0, Tgc, MAX_GATHER_TILES):
                    ns = min(MAX_GATHER_TILES, Tgc - s0)
                    nc.gpsimd.dma_gather(
                        out_ap=gbuf[:, s0 * C:(s0 + ns) * C].rearrange(
                            "p (t e) -> p t e", e=C),
                        in_ap=self.src_views[ch],
                        idxs_ap=idxs[:, (coff + s0) * 8:(coff + s0 + ns) * 8],
                        num_idxs=ns * P,
                        num_idxs_reg=ns * P,
                        elem_size=C,
                        queue_num=qrr % NQUEUES,
                    )
                    qrr += 1
                nc.vector.tensor_tensor(
                    out=msel[:, :Tgc * P],
                    in0=dls[:, coff:coff + Tgc].unsqueeze(2).to_broadcast([P, Tgc, P]),
                    in1=self.iota[:].unsqueeze(1).to_broadcast([P, Tgc, P]),
                    op=mybir.AluOpType.is_equal)
                nc.vector.tensor_tensor(
                    out=msel[:, :Tgc * P],
                    in0=msel[:, :Tgc * P],
                    in1=vls[:, coff:coff + Tgc].unsqueeze(2).to_broadcast([P, Tgc, P]),
                    op=mybir.AluOpType.mult)
                t0 = 0
                for bi, b in enumerate(gb):
                    tb = int(self.T[b, ch])
                    last_ch = max(c for c in range(self.nch) if self.T[b, c] > 0)
                    for t in range(t0, t0 + tb):
                        nc.tensor.matmul(
                            acc[:, bi * P:(bi + 1) * P],
                            lhsT=gbuf[:, t * C:(t + 1) * C],
                            rhs=msel[:, t * P:(t + 1) * P],
                            start=False,
                            stop=(ch == last_ch) and (t == t0 + tb - 1),
                        )
                    t0 += tb
                coff += Tgc
            post(acc, gb)
            tile_off += Tg


def build_program(SA, SB, SD, debug_dumps=False, passes=1):
    nc = bacc.Bacc("TRN2", target_bir_lowering=False, debug=False,
                   num_devices=NCORES, num_swdge_queues=NQUEUES)

    def din(name, shape, dt):
        return nc.dram_tensor(name, shape, dt, kind="ExternalInput").ap()

    xt = din("xt", [XROWS, C], F16)
    w1a = din("w1a", [C, C], F16)
    w1b = din("w1b", [C, C], F16)
    w2a = din("w2a", [C, C], F16)
    w2b = din("w2b", [C, C], F16)
    sA_idx = din("sA_idx", [P, SA["total"] * 8], I16)
    sA_dl = din("sA_dl", [P, SA["total"]], F16)
    sA_vl = din("sA_vl", [P, SA["total"]], F16)
    sB_idx = din("sB_idx", [P, SB["total"] * 8], I16)
    sB_dl = din("sB_dl", [P, SB["total"]], F16)
    sB_vl = din("sB_vl", [P, SB["total"]], F16)
    sD_idx = din("sD_idx", [P, SD["total"] * 8], I16)
    sD_dl = din("sD_dl", [P, SD["total"]], F16)
    sD_vl = din("sD_vl", [P, SD["total"]], F16)
    out = nc.dram_tensor("out", [NSHARD, C], F32, kind="ExternalOutput").ap()
    z1dump = z2dump = None
    if debug_dumps:
        z1dump = nc.dram_tensor("z1dump", [NPAD, C], F16, kind="ExternalOutput").ap()
        z2dump = nc.dram_tensor("z2dump", [EPAD, C], F16, kind="ExternalOutput").ap()

    with tile.TileContext(nc) as tc:
        with tc.tile_pool(name="const", bufs=1) as cpool, \
             tc.tile_pool(name="sb", bufs=3) as spool, \
             tc.tile_pool(name="acc", bufs=2, space="PSUM") as ppool, \
             tc.tile_pool(name="post", bufs=2, space="PSUM") as ppool2, \
             tc.tile_pool(name="dram", bufs=1, space="DRAM") as dpool:

            pools = (cpool, spool, ppool, ppool2)

            iota = cpool.tile([P, P], F16, name="iota")
            nc.gpsimd.iota(iota[:], pattern=[[1, P]], base=0, channel_multiplier=0,
                           allow_small_or_imprecise_dtypes=True)
            w1a_s = cpool.tile([P, C], F16, name="w1a_s")
            w1b_s = cpool.tile([P, C], F16, name="w1b_s")
            w2a_s = cpool.tile([P, C], F16, name="w2a_s")
            w2b_s = cpool.tile([P, C], F16, name="w2b_s")
            nc.sync.dma_start(out=w1a_s[:], in_=w1a[:])
            nc.sync.dma_start(out=w1b_s[:], in_=w1b[:])
            nc.sync.dma_start(out=w2a_s[:], in_=w2a[:])
            nc.sync.dma_start(out=w2b_s[:], in_=w2b[:])

            zrhs = cpool.tile([P, 4 * P], F16, name="zrhs")
            nc.vector.memset(zrhs[:], 0.0)
            s2a_all = cpool.tile([P, (NSHARD // P) * P], F16, name="s2a_all")
            s2b_all = cpool.tile([P, (NSHARD // P) * P], F16, name="s2b_all")

            z1shard = dpool.tile([NSHARD, C], F16, name="z1shard")
            z2shard = dpool.tile([ESHARD, C], F16, name="z2shard")

            x_views = [xt[ci * CHUNK:(ci + 1) * CHUNK, :] for ci in range(4)]

            def post_sig(wsb, zdst):
                def post(acc, gb):
                    for bi, b in enumerate(gb):
                        sp = spool.tile([P, P], F16, tag="sp", name="sp")
                        nc.vector.tensor_copy(out=sp[:], in_=acc[:, bi * P:(bi + 1) * P])
                        ps2 = ppool2.tile([P, C], F32, tag="post", name="ps2")
                        nc.tensor.matmul(ps2[:], lhsT=sp[:], rhs=wsb[:],
                                         start=True, stop=True)
                        zt = spool.tile([P, C], F16, tag="zt", name="zt")
                        nc.scalar.activation(out=zt[:], in_=ps2[:],
                                             func=mybir.ActivationFunctionType.Sigmoid)
                        nc.sync.dma_start(out=zdst[b * P:(b + 1) * P, :], in_=zt[:])
                return post

            def post_stash(dst_all):
                def post(acc, gb):
                    for bi, b in enumerate(gb):
                        nc.vector.tensor_copy(out=dst_all[:, b * P:(b + 1) * P],
                                              in_=acc[:, bi * P:(bi + 1) * P])
                return post

            for _pass in range(passes):
              # collective outputs: a Shared DRAM tensor may have only one
              # writing instruction, so allocate fresh ones per pass
              z1full = dpool.tile([NPAD, C], F16, name=f"z1full{_pass}",
                                  addr_space="Shared")
              z2full = dpool.tile([EPAD, C], F16, name=f"z2full{_pass}",
                                  addr_space="Shared")
              z1_views = [z1full[ci * CHUNK:min((ci + 1) * CHUNK, NPAD), :]
                          for ci in range(4)]
              z2_views = [z2full[ci * CHUNK:min((ci + 1) * CHUNK, EPAD), :]
                          for ci in range(7)]
              # Phase A: z1 = sigmoid((A@x) @ W1_00), node shard
              _Phase(nc, pools, "A", SA, sA_idx, sA_dl, sA_vl, x_views,
                   iota, zrhs, tagset="AC").emit(post_sig(w1a_s, z1shard[:]))
              nc.gpsimd.collective_compute(
                "AllGather", mybir.AluOpType.bypass,
                replica_groups=[list(range(NCORES))],
                ins=[z1shard[:]], outs=[z1full[:]])

              # Phase B: z2 = sigmoid((B^T@x) @ W1_01), edge shard
              _Phase(nc, pools, "B", SB, sB_idx, sB_dl, sB_vl, x_views,
                   iota, zrhs, tagset="BD").emit(post_sig(w1b_s, z2shard[:]))
              nc.gpsimd.collective_compute(
                "AllGather", mybir.AluOpType.bypass,
                replica_groups=[list(range(NCORES))],
                ins=[z2shard[:]], outs=[z2full[:]])

              if debug_dumps:
                nc.sync.dma_start(out=z1dump[:], in_=z1full[:])
                nc.sync.dma_start(out=z2dump[:], in_=z2full[:])

              # Phase C: S2a = A @ z1 (node shard), stash
              _Phase(nc, pools, "C", SA, sA_idx, sA_dl, sA_vl, z1_views,
                     iota, zrhs, tagset="AC").emit(post_stash(s2a_all))
              # Phase D: S2b = B @ z2 (node shard), stash
              _Phase(nc, pools, "D", SD, sD_idx, sD_dl, sD_vl, z2_views,
                     iota, zrhs, tagset="BD").emit(post_stash(s2b_all))

              # Merge: out = sigmoid(S2a @ W2_00 + S2b @ W2_10)
              for b in range(NSHARD // P):
                  psm = ppool2.tile([P, C], F32, tag="post", name="psm")
                  nc.tensor.matmul(psm[:], lhsT=s2a_all[:, b * P:(b + 1) * P],
                                   rhs=w2a_s[:], start=True, stop=False)
                  nc.tensor.matmul(psm[:], lhsT=s2b_all[:, b * P:(b + 1) * P],
                                   rhs=w2b_s[:], start=False, stop=True)
                  ot = spool.tile([P, C], F32, tag="ot", name="ot")
                  nc.scalar.activation(out=ot[:], in_=psm[:],
                                       func=mybir.ActivationFunctionType.Sigmoid)
                  nc.sync.dma_start(out=out[b * P:(b + 1) * P, :], in_=ot[:])
    nc.compile()
    return nc


# ---------------------------------------------------------------- driver

def prepare(inputs):
    x = np.asarray(inputs["x"], np.float32)
    adj_rows = np.asarray(inputs["adj_rows"])
    adj_cols = np.asarray(inputs["adj_cols"])
    adj_vals = np.asarray(inputs["adj_vals"], np.float32)
    inc_rows = np.asarray(inputs["inc_rows"])
    inc_cols = np.asarray(inputs["inc_cols"])
    inc_vals = np.asarray(inputs["inc_vals"], np.float32)

    SA = _prep_phase(adj_rows, adj_cols, adj_vals, NSHARD, NSHARD // P, 4, 4)
    SB = _prep_phase(inc_cols, inc_rows, inc_vals, ESHARD, ESHARD // P, 4, 4)
    SD = _prep_phase(inc_rows, inc_cols, inc_vals, NSHARD, NSHARD // P, 7, 4)
    _LAST["streams"] = (SA, SB, SD)

    xt = np.zeros((XROWS, C), np.float16)
    xt[:NODES] = x.astype(np.float16)

    nc = build_program(SA, SB, SD)

    in_maps = []
    for k in range(NCORES):
        in_maps.append({
            "xt": xt,
            "w1a": np.asarray(inputs["W1_00"], np.float32).astype(np.float16),
            "w1b": np.asarray(inputs["W1_01"], np.float32).astype(np.float16),
            "w2a": np.asarray(inputs["W2_00"], np.float32).astype(np.float16),
            "w2b": np.asarray(inputs["W2_10"], np.float32).astype(np.float16),
            "sA_idx": SA["idx16"][k], "sA_dl": SA["dl16"][k], "sA_vl": SA["vl16"][k],
            "sB_idx": SB["idx16"][k], "sB_dl": SB["dl16"][k], "sB_vl": SB["vl16"][k],
            "sD_idx": SD["idx16"][k], "sD_dl": SD["dl16"][k], "sD_vl": SD["vl16"][k],
        })

    def assemble(results):
        full = np.concatenate([results[k]["out"] for k in range(NCORES)], axis=0)
        return full[:NODES]

    return nc, in_maps, assemble


_LAST = {}


def kernel(**inputs):
    nc, in_maps, assemble = prepare(inputs)
    from concourse.bass_utils import run_bass_kernel_spmd
    res = run_bass_kernel_spmd(nc, in_maps, core_ids=list(range(NCORES)))
    _LAST["nc"] = nc
    _LAST["in_maps"] = in_maps
    _LAST["assemble"] = assemble
    return assemble(res.results)
